# revision 38
# baseline (speedup 1.0000x reference)
"""AlignUniform loss kernel for Trainium2 (8 NeuronCores, SPMD) — v2.

Math:
  qn = q / ||q||, kn = k / ||k||         (row-wise L2 normalize)
  align = mean_i ||qn_i - kn_i||^2 = 2 - 2*mean_i <qn_i, kn_i>
  lunif(x) = log( sum_{i<j} exp(4*<x_i,x_j> - 4) / npairs )   (unit-norm rows)
  out = align + (lunif(qn) + lunif(kn)) / 2

Sharding: the strict-upper pairwise sum is decomposed into 512x512 blocks of
the NxN gram matrix; each of the 8 cores covers 17 blocks (2 diagonal + 15
off-diagonal) via the rotation pairing, with inputs host-gathered so the
compiled program is SPMD-identical on every core.

v2 layout strategy: the host stages BOTH a transposed [D, rows] bf16 copy
(matmul operand layout — no on-device transposes at all) and a natural
[rows, D] bf16 copy (row-sumsq layout, tiled so each partition holds a
contiguous row range).  Device pipeline per chunk of rows:
  sumsq (DVE/GpSimd squares + fold-tree) -> rsqrt (DVE magic-Newton) ->
  flatten rn to a [1, n] row (tiny DMA) -> broadcast to [128, n] (GpSimd) ->
  normalize the transposed copy (DVE bf16 2x) -> gram matmuls (PE bf16) ->
  exp + reduce.
The exp of the 34 [128,2048] PSUM unit tiles is split across TWO engines:
~20 units on ACT (table exp, fused accumulate) and ~14 units on DVE via a
Schraudolph-style bit-trick exp (one tensor_scalar: bf16 bit pattern =
int16(s*738.66 + B)); those bf16 tiles are DMA'd to DRAM and summed on the
host (part of the unshard/all-reduce step).  The align term is one fused
multiply-reduce over the normalized slot-0/1 columns (each global row block
is covered exactly once across the 8 cores).
"""

import functools

import numpy as np

import concourse.bacc as bacc
import concourse.mybir as mybir
import concourse.tile as tile

# ----------------------------------------------------------------------------
# Problem constants (hardcoded per harness contract).
N = 8192
D = 128
NCORES = 8
NB = 16           # row blocks of the full N
BLK = 512
NSLOT = 11        # gathered blocks per core
GROWS = NSLOT * BLK   # 5632 gathered rows per core per tensor

# unit list: (row_slot, col_slot, is_diag) -- identical on every core.
UNITS = (
    [(0, 0, True), (1, 1, True)]
    + [(0, r, False) for r in range(1, 8)]
    + [(1, 1 + r, False) for r in range(1, 8)]
    + [(10, 9, False)]
)
NU = len(UNITS)  # 17

# chunk pipeline: (row0, row1, nat tiles per partition)
CHUNKS = [(0, 1024, 8), (1024, 3072, 16), (3072, 5632, 20)]
# ssq/rn16 compact col layout [128, 88]: per chunk, q seg then k seg
SSQ_SEG = {
    (0, 0): (0, 8), (1, 0): (8, 16),
    (0, 1): (16, 32), (1, 1): (32, 48),
    (0, 2): (48, 68), (1, 2): (68, 88),
}

# wave g = units whose largest slot falls inside chunk g's slots
WAVES = [[0, 1, 2], [3, 4, 5, 6, 9, 10, 11, 12], [7, 8, 13, 14, 15, 16]]
# 9 units take the DVE bit-exp path (offdiag only); rest go to ACT.
# Wave A stays all-ACT (the DVE is busy with the chunk-B/C chains then);
# the DVE share concentrates in waves B/C where the chains are done.
DVE_SET = {
    (0, 4), (1, 4), (0, 10), (1, 10),
    (0, 14), (1, 14), (0, 16), (1, 16), (0, 8),
}
# rn-broadcast pieces per chunk (PSUM outer-product tiles are <= 2048 wide)
BCAST_PIECES = [[(0, 1024)], [(1024, 3072)], [(3072, 5120), (5120, 5632)]]

# global schedule: (ti, u, kind); kind: 0 = ACT exp, 1 = DVE schraudolph
UNIT_SCHED = []
for _w in WAVES:
    for _u in _w:
        for _ti in range(2):
            UNIT_SCHED.append((_ti, _u, 1 if (_ti, _u) in DVE_SET else 0))
ACT_COL = {}
DVE_IDX = {}
for _ti, _u, _k in UNIT_SCHED:
    if _k == 0:
        ACT_COL[(_ti, _u)] = len(ACT_COL)
    else:
        DVE_IDX[(_ti, _u)] = len(DVE_IDX)
N_ACT = len(ACT_COL)   # 20
N_DVE = len(DVE_IDX)   # 14
ALIGN_COL = N_ACT      # accs col for the align accumulate
ACC_COLS = N_ACT + 1

# Schraudolph constants: bf16 bits of exp(4s-4) ~= int16(s*A + B).
# B assumes round-to-nearest fp32->int16 conversion and includes the
# arithmetic-mean-preserving correction sigma=log2(E[(1+f)2^-f])=0.05756.
SCH_A = 738.65988
SCH_B = 16256.0 - 738.65988 - 128.0 * 0.057567


DEBUG_DISABLE: set = set()  # bisect switches: gpsq, pbcast, ttr, schdma, schop


def _core_blocks(c: int) -> list[int]:
    """Row-block indices gathered for core c, slot order 0..10."""
    return [(2 * c + s) % NB for s in range(9)] + [(c + 8) % NB, c]


# ----------------------------------------------------------------------------
# Workaround: this walrus build rejects >1 semaphore wait per instruction, but
# TileContext's stock exit drain carries one wait per active proc.  Split it
# into one single-wait drain per proc.
def _apply_tile_exit_patch():
    import re

    import bass_rust
    from concourse.vector_clock import ScopedClock

    if getattr(tile.TileContext, "_drain_split_patch", False):
        return

    def _drain_and_barrier(self, tick_clock, wait_clock):
        nc = self.nc
        ticks = [int(s) for s in re.findall(r"\d+", repr(tick_clock.global_clock))]
        for p, t in ((p, t) for p, t in enumerate(ticks) if t > 0):
            vc = bass_rust.VectorClock()
            vc.require_at_least(p, t)
            d = nc.sync.drain()
            wait_clock.add_sem_waits(d.ins, ScopedClock({None: vc}))
        nc.all_engine_barrier()
        assert self.sems is not None
        popped = nc._tile_sem_poison_stack.pop()
        assert popped is self._sem_poison
        nc.clear_and_free_semaphores(list(self.sems.allocated().values()))
        nc.all_engine_barrier()

    tile.TileContext._drain_and_barrier = _drain_and_barrier
    tile.TileContext._drain_split_patch = True


# ----------------------------------------------------------------------------
def _emit(nc, tc, ctx, qt_d, kt_d, qn_d, kn_d, out_d, sch_d):
    f32 = mybir.dt.float32
    bf16 = mybir.dt.bfloat16
    i16 = mybir.dt.int16
    u32 = mybir.dt.uint32
    AF = mybir.ActivationFunctionType
    ALU = mybir.AluOpType

    big = ctx.enter_context(tc.tile_pool(name="big", bufs=1))
    scratch = ctx.enter_context(tc.tile_pool(name="scratch", bufs=2))
    psp = ctx.enter_context(tc.tile_pool(name="ps", bufs=2, space="PSUM"))

    t_d = (qt_d, kt_d)
    n_d = (qn_d, kn_d)

    xt = [big.tile([128, GROWS], bf16, tag=f"xt{ti}", name=f"xt{ti}") for ti in range(2)]
    xtn = [big.tile([128, GROWS], bf16, tag=f"xtn{ti}", name=f"xtn{ti}") for ti in range(2)]
    rnrow = [big.tile([1, GROWS], bf16, tag=f"rnrow{ti}", name=f"rnrow{ti}") for ti in range(2)]
    ones1 = big.tile([1, 512], bf16, tag="ones1")
    nc.vector.memset(ones1, 1.0)
    nat = [
        [big.tile([128, t, D], bf16, tag=f"nat{ti}_{g}", name=f"nat{ti}_{g}") for g, (_, _, t) in enumerate(CHUNKS)]
        for ti in range(2)
    ]
    ssq = big.tile([128, 88], f32, tag="ssq")
    rn = big.tile([128, 88], f32, tag="rn")
    rn16 = big.tile([128, 88], bf16, tag="rn16")
    accs = big.tile([128, ACC_COLS], f32, tag="accs")
    biasm4 = big.tile([128, 1], f32, tag="biasm4")
    nc.vector.memset(biasm4, -4.0)
    magic = big.tile([128, 1], u32, tag="magic")
    nc.vector.memset(magic, 0x5F3759DF)

    # ---- input DMAs, chunk A first so its chain starts early; halve each
    # chunk-A transfer so it spreads over more queues.
    for g, (r0, r1, t) in enumerate(CHUNKS):
        for ti in range(2):
            if g == 0:
                rm = (r0 + r1) // 2
                nc.sync.dma_start(
                    nat[ti][g][0:64, :, :],
                    n_d[ti][r0:rm].rearrange("(p t) d -> p t d", p=64),
                )
                nc.sync.dma_start(
                    nat[ti][g][64:128, :, :],
                    n_d[ti][rm:r1].rearrange("(p t) d -> p t d", p=64),
                )
                nc.sync.dma_start(xt[ti][:, r0:rm], t_d[ti][:, r0:rm])
                nc.sync.dma_start(xt[ti][:, rm:r1], t_d[ti][:, rm:r1])
            else:
                # nat first (it gates the chunk's sumsq chain), split in half
                tm = t // 2
                src = n_d[ti][r0:r1].rearrange("(p t) d -> p t d", p=128)
                nc.sync.dma_start(nat[ti][g][:, 0:tm, :], src[:, 0:tm, :])
                nc.sync.dma_start(nat[ti][g][:, tm:t, :], src[:, tm:t, :])
                nc.sync.dma_start(xt[ti][:, r0:r1], t_d[ti][:, r0:r1])

    def sumsq_chunk(ti, g, square_engine):
        """squares + fold tree + reduce -> ssq segment (compact f32)."""
        _, _, t = CHUNKS[g]
        s0, s1 = SSQ_SEG[(ti, g)]
        sq = scratch.tile([128, t, D], bf16, tag=f"sq{g}", name=f"sq{ti}_{g}")
        square_engine.tensor_tensor(sq[:], nat[ti][g][:], nat[ti][g][:], ALU.mult)
        f1 = scratch.tile([128, t, 64], bf16, tag=f"f1{g}", name=f"f1{ti}_{g}")
        nc.vector.tensor_tensor(f1[:], sq[:, :, 0:64], sq[:, :, 64:128], ALU.add)
        f2 = scratch.tile([128, t, 32], bf16, tag=f"f2{g}", name=f"f2{ti}_{g}")
        nc.vector.tensor_tensor(f2[:], f1[:, :, 0:32], f1[:, :, 32:64], ALU.add)
        nc.vector.tensor_reduce(ssq[:, s0:s1], f2[:], mybir.AxisListType.X, ALU.add)

    def newton_seg(c0, c1):
        """rn = 1/sqrt(ssq) on ssq cols [c0, c1): magic + 1 Newton step."""
        w = c1 - c0
        x = ssq[:, c0:c1]
        y = rn[:, c0:c1]
        yu = y.bitcast(u32)
        hx = scratch.tile([128, w], f32, tag="nr_hx")
        tmp = scratch.tile([128, w], f32, tag="nr_tmp")
        nc.vector.tensor_scalar(yu, x.bitcast(u32), 1, None, op0=ALU.logical_shift_right)
        nc.vector.tensor_tensor(yu, magic[:, 0:1].to_broadcast((128, w)), yu, ALU.subtract)
        nc.vector.tensor_scalar(hx[:], x, 0.5, None, op0=ALU.mult)
        for _ in range(1):
            nc.vector.tensor_tensor(tmp[:], y, y, ALU.mult)
            nc.vector.tensor_tensor(tmp[:], tmp[:], hx[:], ALU.mult)
            nc.vector.tensor_scalar(tmp[:], tmp[:], -1.0, 1.5, op0=ALU.mult, op1=ALU.add)
            nc.vector.tensor_tensor(y, y, tmp[:], ALU.mult)
        nc.vector.tensor_copy(rn16[:, c0:c1], y)

    def spread_chunk(ti, g):
        """compact rn16 -> [1,n] row -> PE outer-product broadcast into PSUM
        -> normalize xt straight from PSUM.  (GpSimd partition_broadcast is
        avoided -- GpSimd tensor ops starve the DVE on the shared SBUF port;
        stride-0-source DMAs degenerate to per-element descriptors; DMA
        doubling chains cost ~3us serial latency per hop.)"""
        r0, r1, t = CHUNKS[g]
        s0, s1 = SSQ_SEG[(ti, g)]
        # issue via the Activation DGE: its queue rings are empty, while the
        # SP rings hold megabytes of queued input loads that would delay this
        # tiny latency-critical transfer by 10+us.
        nc.scalar.dma_start(
            rnrow[ti][0:1, r0:r1].rearrange("o (p t) -> o p t", p=128),
            rn16[:, s0:s1],
        )
        for c0, c1 in BCAST_PIECES[g]:
            w = c1 - c0
            rnp = psp.tile([128, 2048], f32, tag="ps", name=f"rnp{ti}_{g}_{c0}")
            for m0 in range(0, w, 512):
                m1 = min(m0 + 512, w)
                nc.tensor.matmul(
                    rnp[:, m0:m1],
                    lhsT=ones1[:, 0:128],
                    rhs=rnrow[ti][0:1, c0 + m0 : c0 + m1],
                    start=True,
                    stop=True,
                )
            nc.vector.tensor_tensor(
                xtn[ti][:, c0:c1], xt[ti][:, c0:c1], rnp[:, 0:w], ALU.mult
            )

    def emit_unit(ti, u):
        rs, cs, _ = UNITS[u]
        ps = psp.tile([128, 2048], f32, tag="ps", name=f"ps{ti}_{u}")
        for m in range(4):
            nc.tensor.matmul(
                ps[:, 512 * m : 512 * (m + 1)],
                lhsT=xtn[ti][:, BLK * rs + 128 * m : BLK * rs + 128 * (m + 1)],
                rhs=xtn[ti][:, BLK * cs : BLK * (cs + 1)],
                start=True,
                stop=True,
            )
        if (ti, u) in ACT_COL:
            col = ACT_COL[(ti, u)]
            ad = scratch.tile([128, 2048], bf16, tag="actdump")
            nc.scalar.activation(
                ad[:], ps[:], AF.Exp, bias=biasm4[:], scale=4.0,
                accum_out=accs[:, col : col + 1],
            )
        else:
            idx = DVE_IDX[(ti, u)]
            sch = scratch.tile([128, 2048], i16, tag="sch")
            if "schop" in DEBUG_DISABLE:
                nc.vector.tensor_scalar(
                    sch[:].bitcast(bf16), ps[:], 1.0, None, op0=ALU.mult
                )
            else:
                # 4 pieces: each starts as soon as its matmul lands (subtile
                # deps) and the PSUM slot frees right after the last piece.
                for m in range(4):
                    nc.vector.tensor_scalar(
                        sch[:, 512 * m : 512 * (m + 1)],
                        ps[:, 512 * m : 512 * (m + 1)],
                        SCH_A, SCH_B, op0=ALU.mult, op1=ALU.add,
                    )
            if "schdma" not in DEBUG_DISABLE:
                nc.sync.dma_start(sch_d[idx], sch[:].bitcast(bf16))

    # ---- PE warm-up: dummy K=1 matmuls reading the freshly-landed xt tile
    # keep HAM busy from the moment inputs arrive until the first real grams,
    # so those run at the unthrottled clock.
    dps = psp.tile([128, 2048], f32, tag="ps", name="dummyps")
    for m in range(12):
        nc.tensor.matmul(
            dps[:, 512 * (m % 4) : 512 * (m % 4 + 1)],
            lhsT=ones1[:, 0:128],
            rhs=xt[0][0:1, 0:512],
            start=True,
            stop=True,
        )

    # ---- chunk A, per tensor: fastest possible path to the first exps
    for ti in range(2):
        sumsq_chunk(ti, 0, nc.vector)
        newton_seg(*SSQ_SEG[(ti, 0)])
        spread_chunk(ti, 0)
        for u in WAVES[0]:
            emit_unit(ti, u)

    # ---- chunk B chain, then wave B first half (chunk C chain mid-wave)
    for ti in range(2):
        sumsq_chunk(ti, 1, nc.vector)
    newton_seg(16, 48)
    for ti in range(2):
        spread_chunk(ti, 1)

    WB = [(0, 3), (1, 3), (0, 4), (0, 5), (1, 5), (1, 4), (0, 6), (1, 6),
          (0, 10), (0, 9), (1, 9), (1, 10), (0, 11), (1, 11), (0, 12), (1, 12)]
    for ti, u in WB[:8]:
        emit_unit(ti, u)

    for ti in range(2):
        sumsq_chunk(ti, 2, nc.vector)
    newton_seg(48, 88)
    for ti in range(2):
        spread_chunk(ti, 2)

    for ti, u in WB[8:]:
        emit_unit(ti, u)

    # align term: sum <qn_i, kn_i> over slots 0-1 rows (once per row globally)
    aldump = scratch.tile([128, 1024], bf16, tag="aldump")
    nc.vector.tensor_tensor(aldump[:], xtn[0][:, 0:1024], xtn[1][:, 0:1024], ALU.mult)
    nc.vector.tensor_reduce(
        accs[:, ALIGN_COL : ALIGN_COL + 1], aldump[:], mybir.AxisListType.X, ALU.add
    )

    WC = [(0, 7), (0, 8), (1, 7), (0, 14), (0, 13), (1, 14), (1, 13),
          (0, 16), (0, 15), (1, 16), (1, 15), (1, 8)]
    for ti, u in WC:
        emit_unit(ti, u)

    nc.sync.dma_start(out_d[:], accs[:])


@functools.lru_cache(maxsize=1)
def _build():
    from contextlib import ExitStack

    _apply_tile_exit_patch()
    nc = bacc.Bacc("TRN2", target_bir_lowering=False, debug=False, num_devices=NCORES)
    f32 = mybir.dt.float32
    bf16 = mybir.dt.bfloat16
    i16 = mybir.dt.int16
    qt = nc.dram_tensor("qt", [D, GROWS], bf16, kind="ExternalInput")
    kt = nc.dram_tensor("kt", [D, GROWS], bf16, kind="ExternalInput")
    qn = nc.dram_tensor("qn", [GROWS, D], bf16, kind="ExternalInput")
    kn = nc.dram_tensor("kn", [GROWS, D], bf16, kind="ExternalInput")
    out = nc.dram_tensor("out", [128, ACC_COLS], f32, kind="ExternalOutput")
    sch = nc.dram_tensor("sch", [N_DVE, 128, 2048], bf16, kind="ExternalOutput")
    with tile.TileContext(nc) as tc, ExitStack() as ctx:
        _emit(nc, tc, ctx, qt.ap(), kt.ap(), qn.ap(), kn.ap(), out.ap(), sch.ap())
    nc.compile()
    return nc


def _bf16(x: np.ndarray):
    import ml_dtypes

    return np.ascontiguousarray(x).astype(ml_dtypes.bfloat16)


def _stage(x: np.ndarray, c: int):
    """Gather core c's row blocks; return (transposed bf16, natural bf16)."""
    g = np.concatenate([x[BLK * b : BLK * (b + 1)] for b in _core_blocks(c)])
    return _bf16(g.T), _bf16(g)


def run_device(q: np.ndarray, k: np.ndarray, **run_kwargs):
    """Compile + run on the 8 cores; returns BassKernelResults."""
    from concourse.bass_utils import run_bass_kernel_spmd

    nc = _build()
    in_maps = []
    for c in range(NCORES):
        qt, qn = _stage(q, c)
        kt, kn = _stage(k, c)
        in_maps.append({"qt": qt, "kt": kt, "qn": qn, "kn": kn})
    return run_bass_kernel_spmd(nc, in_maps, core_ids=list(range(NCORES)), **run_kwargs)


def reduce_outputs(outs: list) -> np.float32:
    """Host-side gather/unshard: fold per-core accumulators into the scalar."""
    npairs = N * (N - 1) / 2.0
    diag = [0.0, 0.0]
    off = [0.0, 0.0]
    align_dot = 0.0
    for c in range(NCORES):
        acc = outs[c]["out"].astype(np.float64)
        for (ti, u), col in ACT_COL.items():
            s = acc[:, col].sum()
            if UNITS[u][2]:
                diag[ti] += s
            else:
                off[ti] += s
        align_dot += acc[:, ALIGN_COL].sum()
        schf = np.asarray(outs[c]["sch"]).astype(np.float64)
        for (ti, u), idx in DVE_IDX.items():
            off[ti] += schf[idx].sum()
    terms = [np.log((off[ti] + (diag[ti] - N) / 2.0) / npairs) for ti in range(2)]
    align = 2.0 - 2.0 * align_dot / N
    return np.float32(align + (terms[0] + terms[1]) / 2.0)


def kernel(q: np.ndarray, k: np.ndarray) -> np.ndarray:
    res = run_device(q, k)
    return np.asarray(reduce_outputs(res.results), dtype=np.float32)


# revision 39
# speedup vs baseline: 1.0951x; 1.0951x over previous
"""AlignUniform loss kernel for Trainium2 (8 NeuronCores, SPMD) — v2.

Math:
  qn = q / ||q||, kn = k / ||k||         (row-wise L2 normalize)
  align = mean_i ||qn_i - kn_i||^2 = 2 - 2*mean_i <qn_i, kn_i>
  lunif(x) = log( sum_{i<j} exp(4*<x_i,x_j> - 4) / npairs )   (unit-norm rows)
  out = align + (lunif(qn) + lunif(kn)) / 2

Sharding: the strict-upper pairwise sum is decomposed into 512x512 blocks of
the NxN gram matrix; each of the 8 cores covers 17 blocks (2 diagonal + 15
off-diagonal) via the rotation pairing, with inputs host-gathered so the
compiled program is SPMD-identical on every core.

v2 layout strategy: the host stages BOTH a transposed [D, rows] bf16 copy
(matmul operand layout — no on-device transposes at all) and a natural
[rows, D] bf16 copy (row-sumsq layout, tiled so each partition holds a
contiguous row range).  Device pipeline per chunk of rows:
  sumsq (DVE/GpSimd squares + fold-tree) -> rsqrt (DVE magic-Newton) ->
  flatten rn to a [1, n] row (tiny DMA) -> broadcast to [128, n] (GpSimd) ->
  normalize the transposed copy (DVE bf16 2x) -> gram matmuls (PE bf16) ->
  exp + reduce.
The exp of the 34 [128,2048] PSUM unit tiles is split across TWO engines:
~20 units on ACT (table exp, fused accumulate) and ~14 units on DVE via a
Schraudolph-style bit-trick exp (one tensor_scalar: bf16 bit pattern =
int16(s*738.66 + B)); those bf16 tiles are DMA'd to DRAM and summed on the
host (part of the unshard/all-reduce step).  The align term is one fused
multiply-reduce over the normalized slot-0/1 columns (each global row block
is covered exactly once across the 8 cores).
"""

import functools

import numpy as np

import concourse.bacc as bacc
import concourse.mybir as mybir
import concourse.tile as tile

# ----------------------------------------------------------------------------
# Problem constants (hardcoded per harness contract).
N = 8192
D = 128
NCORES = 8
NB = 16           # row blocks of the full N
BLK = 512
NSLOT = 11        # gathered blocks per core
GROWS = NSLOT * BLK   # 5632 gathered rows per core per tensor

# unit list: (row_slot, col_slot, is_diag) -- identical on every core.
UNITS = (
    [(0, 0, True), (1, 1, True)]
    + [(0, r, False) for r in range(1, 8)]
    + [(1, 1 + r, False) for r in range(1, 8)]
    + [(10, 9, False)]
)
NU = len(UNITS)  # 17

# chunk pipeline: (row0, row1, nat tiles per partition)
CHUNKS = [(0, 1024, 8), (1024, 3072, 16), (3072, 5632, 20)]
# ssq/rn16 compact col layout [128, 88]: per chunk, q seg then k seg
SSQ_SEG = {
    (0, 0): (0, 8), (1, 0): (8, 16),
    (0, 1): (16, 32), (1, 1): (32, 48),
    (0, 2): (48, 68), (1, 2): (68, 88),
}

# wave g = units whose largest slot falls inside chunk g's slots
WAVES = [[0, 1, 2], [3, 4, 5, 6, 9, 10, 11, 12], [7, 8, 13, 14, 15, 16]]
# 9 units take the DVE bit-exp path (offdiag only); rest go to ACT.
# Wave A stays all-ACT (the DVE is busy with the chunk-B/C chains then);
# the DVE share concentrates in waves B/C where the chains are done.
DVE_SET = {
    (0, 4), (1, 4), (0, 10), (1, 10),
    (0, 14), (1, 14), (0, 16), (1, 16), (0, 8),
}
# rn-broadcast pieces per chunk (PSUM outer-product tiles are <= 2048 wide)
BCAST_PIECES = [[(0, 1024)], [(1024, 3072)], [(3072, 5120), (5120, 5632)]]

# global schedule: (ti, u, kind); kind: 0 = ACT exp, 1 = DVE schraudolph
UNIT_SCHED = []
for _w in WAVES:
    for _u in _w:
        for _ti in range(2):
            UNIT_SCHED.append((_ti, _u, 1 if (_ti, _u) in DVE_SET else 0))
ACT_COL = {}
DVE_IDX = {}
for _ti, _u, _k in UNIT_SCHED:
    if _k == 0:
        ACT_COL[(_ti, _u)] = len(ACT_COL)
    else:
        DVE_IDX[(_ti, _u)] = len(DVE_IDX)
N_ACT = len(ACT_COL)   # 20
N_DVE = len(DVE_IDX)   # 14
ALIGN_COL = N_ACT      # accs col for the align accumulate
ACC_COLS = N_ACT + 1

# Schraudolph constants: bf16 bits of exp(4s-4) ~= int16(s*A + B).
# B assumes round-to-nearest fp32->int16 conversion and includes the
# arithmetic-mean-preserving correction sigma=log2(E[(1+f)2^-f])=0.05756.
SCH_A = 738.65988
SCH_B = 16256.0 - 738.65988 - 128.0 * 0.057567


DEBUG_DISABLE: set = set()  # bisect switches: gpsq, pbcast, ttr, schdma, schop


def _core_blocks(c: int) -> list[int]:
    """Row-block indices gathered for core c, slot order 0..10."""
    return [(2 * c + s) % NB for s in range(9)] + [(c + 8) % NB, c]


# ----------------------------------------------------------------------------
# Workaround: this walrus build rejects >1 semaphore wait per instruction, but
# TileContext's stock exit drain carries one wait per active proc.  Split it
# into one single-wait drain per proc.
def _apply_tile_exit_patch():
    import re

    import bass_rust
    from concourse.vector_clock import ScopedClock

    if getattr(tile.TileContext, "_drain_split_patch", False):
        return

    def _drain_and_barrier(self, tick_clock, wait_clock):
        nc = self.nc
        ticks = [int(s) for s in re.findall(r"\d+", repr(tick_clock.global_clock))]
        for p, t in ((p, t) for p, t in enumerate(ticks) if t > 0):
            vc = bass_rust.VectorClock()
            vc.require_at_least(p, t)
            d = nc.sync.drain()
            wait_clock.add_sem_waits(d.ins, ScopedClock({None: vc}))
        nc.all_engine_barrier()
        assert self.sems is not None
        popped = nc._tile_sem_poison_stack.pop()
        assert popped is self._sem_poison
        nc.clear_and_free_semaphores(list(self.sems.allocated().values()))
        nc.all_engine_barrier()

    tile.TileContext._drain_and_barrier = _drain_and_barrier
    tile.TileContext._drain_split_patch = True


# ----------------------------------------------------------------------------
def _emit(nc, tc, ctx, qt_d, kt_d, qn_d, kn_d, out_d, sch_d):
    f32 = mybir.dt.float32
    bf16 = mybir.dt.bfloat16
    i16 = mybir.dt.int16
    u32 = mybir.dt.uint32
    AF = mybir.ActivationFunctionType
    ALU = mybir.AluOpType

    big = ctx.enter_context(tc.tile_pool(name="big", bufs=1))
    scratch = ctx.enter_context(tc.tile_pool(name="scratch", bufs=2))
    psp = ctx.enter_context(tc.tile_pool(name="ps", bufs=2, space="PSUM"))

    t_d = (qt_d, kt_d)
    n_d = (qn_d, kn_d)

    xt = [big.tile([128, GROWS], bf16, tag=f"xt{ti}", name=f"xt{ti}") for ti in range(2)]
    xtn = [big.tile([128, GROWS], bf16, tag=f"xtn{ti}", name=f"xtn{ti}") for ti in range(2)]
    rnrow = [big.tile([1, GROWS], bf16, tag=f"rnrow{ti}", name=f"rnrow{ti}") for ti in range(2)]
    ones1 = big.tile([1, 512], bf16, tag="ones1")
    nc.vector.memset(ones1, 1.0)
    nat = [
        [big.tile([128, t, D], bf16, tag=f"nat{ti}_{g}", name=f"nat{ti}_{g}") for g, (_, _, t) in enumerate(CHUNKS)]
        for ti in range(2)
    ]
    ssq = big.tile([128, 88], f32, tag="ssq")
    rn = big.tile([128, 88], f32, tag="rn")
    rn16 = big.tile([128, 88], bf16, tag="rn16")
    accs = big.tile([128, ACC_COLS], f32, tag="accs")
    biasm4 = big.tile([128, 1], f32, tag="biasm4")
    nc.vector.memset(biasm4, -4.0)
    magic = big.tile([128, 1], u32, tag="magic")
    nc.vector.memset(magic, 0x5F3759DF)

    # ---- input DMAs, chunk A first so its chain starts early; halve each
    # chunk-A transfer so it spreads over more queues.
    for g, (r0, r1, t) in enumerate(CHUNKS):
        for ti in range(2):
            if g == 0:
                rm = (r0 + r1) // 2
                nc.sync.dma_start(
                    nat[ti][g][0:64, :, :],
                    n_d[ti][r0:rm].rearrange("(p t) d -> p t d", p=64),
                )
                nc.sync.dma_start(
                    nat[ti][g][64:128, :, :],
                    n_d[ti][rm:r1].rearrange("(p t) d -> p t d", p=64),
                )
                nc.sync.dma_start(xt[ti][:, r0:rm], t_d[ti][:, r0:rm])
                nc.sync.dma_start(xt[ti][:, rm:r1], t_d[ti][:, rm:r1])
            else:
                # nat first (it gates the chunk's sumsq chain), split in half
                tm = t // 2
                src = n_d[ti][r0:r1].rearrange("(p t) d -> p t d", p=128)
                nc.sync.dma_start(nat[ti][g][:, 0:tm, :], src[:, 0:tm, :])
                nc.sync.dma_start(nat[ti][g][:, tm:t, :], src[:, tm:t, :])
                nc.sync.dma_start(xt[ti][:, r0:r1], t_d[ti][:, r0:r1])

    def sumsq_chunk(ti, g, square_engine):
        """squares + fold tree + reduce -> ssq segment (compact f32)."""
        _, _, t = CHUNKS[g]
        s0, s1 = SSQ_SEG[(ti, g)]
        sq = scratch.tile([128, t, D], bf16, tag=f"sq{g}", name=f"sq{ti}_{g}")
        square_engine.tensor_tensor(sq[:], nat[ti][g][:], nat[ti][g][:], ALU.mult)
        f1 = scratch.tile([128, t, 64], bf16, tag=f"f1{g}", name=f"f1{ti}_{g}")
        nc.vector.tensor_tensor(f1[:], sq[:, :, 0:64], sq[:, :, 64:128], ALU.add)
        f2 = scratch.tile([128, t, 32], bf16, tag=f"f2{g}", name=f"f2{ti}_{g}")
        nc.vector.tensor_tensor(f2[:], f1[:, :, 0:32], f1[:, :, 32:64], ALU.add)
        nc.vector.tensor_reduce(ssq[:, s0:s1], f2[:], mybir.AxisListType.X, ALU.add)

    def newton_seg(c0, c1):
        """rn = 1/sqrt(ssq) on ssq cols [c0, c1): magic + 1 Newton step."""
        w = c1 - c0
        x = ssq[:, c0:c1]
        y = rn[:, c0:c1]
        yu = y.bitcast(u32)
        hx = scratch.tile([128, w], f32, tag="nr_hx")
        tmp = scratch.tile([128, w], f32, tag="nr_tmp")
        nc.vector.tensor_scalar(yu, x.bitcast(u32), 1, None, op0=ALU.logical_shift_right)
        nc.vector.tensor_tensor(yu, magic[:, 0:1].to_broadcast((128, w)), yu, ALU.subtract)
        nc.vector.tensor_scalar(hx[:], x, 0.5, None, op0=ALU.mult)
        for _ in range(1):
            nc.vector.tensor_tensor(tmp[:], y, y, ALU.mult)
            nc.vector.tensor_tensor(tmp[:], tmp[:], hx[:], ALU.mult)
            nc.vector.tensor_scalar(tmp[:], tmp[:], -1.0, 1.5, op0=ALU.mult, op1=ALU.add)
            nc.vector.tensor_tensor(y, y, tmp[:], ALU.mult)
        nc.vector.tensor_copy(rn16[:, c0:c1], y)

    def spread_chunk(ti, g):
        """compact rn16 -> [1,n] row -> PE outer-product broadcast into PSUM
        -> normalize xt straight from PSUM.  (GpSimd partition_broadcast is
        avoided -- GpSimd tensor ops starve the DVE on the shared SBUF port;
        stride-0-source DMAs degenerate to per-element descriptors; DMA
        doubling chains cost ~3us serial latency per hop.)"""
        r0, r1, t = CHUNKS[g]
        s0, s1 = SSQ_SEG[(ti, g)]
        # issue via the Activation DGE: its queue rings are empty, while the
        # SP rings hold megabytes of queued input loads that would delay this
        # tiny latency-critical transfer by 10+us.
        nc.scalar.dma_start(
            rnrow[ti][0:1, r0:r1].rearrange("o (p t) -> o p t", p=128),
            rn16[:, s0:s1],
        )
        for c0, c1 in BCAST_PIECES[g]:
            w = c1 - c0
            rnp = psp.tile([128, 2048], f32, tag="ps", name=f"rnp{ti}_{g}_{c0}")
            for m0 in range(0, w, 512):
                m1 = min(m0 + 512, w)
                nc.tensor.matmul(
                    rnp[:, m0:m1],
                    lhsT=ones1[:, 0:128],
                    rhs=rnrow[ti][0:1, c0 + m0 : c0 + m1],
                    start=True,
                    stop=True,
                )
            nc.vector.tensor_tensor(
                xtn[ti][:, c0:c1], xt[ti][:, c0:c1], rnp[:, 0:w], ALU.mult
            )

    def emit_unit(ti, u):
        rs, cs, _ = UNITS[u]
        ps = psp.tile([128, 2048], f32, tag="ps", name=f"ps{ti}_{u}")
        for m in range(4):
            nc.tensor.matmul(
                ps[:, 512 * m : 512 * (m + 1)],
                lhsT=xtn[ti][:, BLK * rs + 128 * m : BLK * rs + 128 * (m + 1)],
                rhs=xtn[ti][:, BLK * cs : BLK * (cs + 1)],
                start=True,
                stop=True,
            )
        if (ti, u) in ACT_COL:
            col = ACT_COL[(ti, u)]
            ad = scratch.tile([128, 2048], bf16, tag="actdump")
            nc.scalar.activation(
                ad[:], ps[:], AF.Exp, bias=biasm4[:], scale=4.0,
                accum_out=accs[:, col : col + 1],
            )
        else:
            idx = DVE_IDX[(ti, u)]
            sch = scratch.tile([128, 2048], i16, tag="sch")
            if "schop" in DEBUG_DISABLE:
                nc.vector.tensor_scalar(
                    sch[:].bitcast(bf16), ps[:], 1.0, None, op0=ALU.mult
                )
            else:
                nc.vector.tensor_scalar(
                    sch[:], ps[:], SCH_A, SCH_B, op0=ALU.mult, op1=ALU.add
                )
            if "schdma" not in DEBUG_DISABLE:
                nc.sync.dma_start(sch_d[idx], sch[:].bitcast(bf16))

    # ---- PE warm-up: dummy K=1 matmuls reading the freshly-landed xt tile
    # keep HAM busy from the moment inputs arrive until the first real grams,
    # so those run at the unthrottled clock.
    dps = psp.tile([128, 2048], f32, tag="ps", name="dummyps")
    for m in range(12):
        nc.tensor.matmul(
            dps[:, 512 * (m % 4) : 512 * (m % 4 + 1)],
            lhsT=ones1[:, 0:128],
            rhs=xt[0][0:1, 0:512],
            start=True,
            stop=True,
        )

    # ---- chunk A, per tensor: fastest possible path to the first exps
    for ti in range(2):
        sumsq_chunk(ti, 0, nc.vector)
        newton_seg(*SSQ_SEG[(ti, 0)])
        spread_chunk(ti, 0)
        for u in WAVES[0]:
            emit_unit(ti, u)

    # ---- chunk B chain, then wave B first half (chunk C chain mid-wave)
    for ti in range(2):
        sumsq_chunk(ti, 1, nc.vector)
    newton_seg(16, 48)
    for ti in range(2):
        spread_chunk(ti, 1)

    WB = [(0, 3), (1, 3), (0, 4), (0, 5), (1, 5), (1, 4), (0, 6), (1, 6),
          (0, 10), (0, 9), (1, 9), (1, 10), (0, 11), (1, 11), (0, 12), (1, 12)]
    for ti, u in WB[:8]:
        emit_unit(ti, u)

    for ti in range(2):
        sumsq_chunk(ti, 2, nc.vector)
    newton_seg(48, 88)
    for ti in range(2):
        spread_chunk(ti, 2)

    for ti, u in WB[8:]:
        emit_unit(ti, u)

    # align term: sum <qn_i, kn_i> over slots 0-1 rows (once per row globally)
    aldump = scratch.tile([128, 1024], bf16, tag="aldump")
    nc.vector.tensor_tensor(aldump[:], xtn[0][:, 0:1024], xtn[1][:, 0:1024], ALU.mult)
    nc.vector.tensor_reduce(
        accs[:, ALIGN_COL : ALIGN_COL + 1], aldump[:], mybir.AxisListType.X, ALU.add
    )

    WC = [(0, 7), (0, 8), (1, 7), (0, 14), (0, 13), (1, 14), (1, 13),
          (0, 16), (0, 15), (1, 16), (1, 15), (1, 8)]
    for ti, u in WC:
        emit_unit(ti, u)

    nc.sync.dma_start(out_d[:], accs[:])


@functools.lru_cache(maxsize=1)
def _build():
    from contextlib import ExitStack

    _apply_tile_exit_patch()
    nc = bacc.Bacc("TRN2", target_bir_lowering=False, debug=False, num_devices=NCORES)
    f32 = mybir.dt.float32
    bf16 = mybir.dt.bfloat16
    i16 = mybir.dt.int16
    qt = nc.dram_tensor("qt", [D, GROWS], bf16, kind="ExternalInput")
    kt = nc.dram_tensor("kt", [D, GROWS], bf16, kind="ExternalInput")
    qn = nc.dram_tensor("qn", [GROWS, D], bf16, kind="ExternalInput")
    kn = nc.dram_tensor("kn", [GROWS, D], bf16, kind="ExternalInput")
    out = nc.dram_tensor("out", [128, ACC_COLS], f32, kind="ExternalOutput")
    sch = nc.dram_tensor("sch", [N_DVE, 128, 2048], bf16, kind="ExternalOutput")
    with tile.TileContext(nc) as tc, ExitStack() as ctx:
        _emit(nc, tc, ctx, qt.ap(), kt.ap(), qn.ap(), kn.ap(), out.ap(), sch.ap())
    nc.compile()
    return nc


def _bf16(x: np.ndarray):
    import ml_dtypes

    return np.ascontiguousarray(x).astype(ml_dtypes.bfloat16)


def _stage(x: np.ndarray, c: int):
    """Gather core c's row blocks; return (transposed bf16, natural bf16)."""
    g = np.concatenate([x[BLK * b : BLK * (b + 1)] for b in _core_blocks(c)])
    return _bf16(g.T), _bf16(g)


def run_device(q: np.ndarray, k: np.ndarray, **run_kwargs):
    """Compile + run on the 8 cores; returns BassKernelResults."""
    from concourse.bass_utils import run_bass_kernel_spmd

    nc = _build()
    in_maps = []
    for c in range(NCORES):
        qt, qn = _stage(q, c)
        kt, kn = _stage(k, c)
        in_maps.append({"qt": qt, "kt": kt, "qn": qn, "kn": kn})
    return run_bass_kernel_spmd(nc, in_maps, core_ids=list(range(NCORES)), **run_kwargs)


def reduce_outputs(outs: list) -> np.float32:
    """Host-side gather/unshard: fold per-core accumulators into the scalar."""
    npairs = N * (N - 1) / 2.0
    diag = [0.0, 0.0]
    off = [0.0, 0.0]
    align_dot = 0.0
    for c in range(NCORES):
        acc = outs[c]["out"].astype(np.float64)
        for (ti, u), col in ACT_COL.items():
            s = acc[:, col].sum()
            if UNITS[u][2]:
                diag[ti] += s
            else:
                off[ti] += s
        align_dot += acc[:, ALIGN_COL].sum()
        schf = np.asarray(outs[c]["sch"]).astype(np.float64)
        for (ti, u), idx in DVE_IDX.items():
            off[ti] += schf[idx].sum()
    terms = [np.log((off[ti] + (diag[ti] - N) / 2.0) / npairs) for ti in range(2)]
    align = 2.0 - 2.0 * align_dot / N
    return np.float32(align + (terms[0] + terms[1]) / 2.0)


def kernel(q: np.ndarray, k: np.ndarray) -> np.ndarray:
    res = run_device(q, k)
    return np.asarray(reduce_outputs(res.results), dtype=np.float32)


# revision 44
# speedup vs baseline: 1.1406x; 1.0416x over previous
"""AlignUniform loss kernel for Trainium2 (8 NeuronCores, SPMD) — v2.

Math:
  qn = q / ||q||, kn = k / ||k||         (row-wise L2 normalize)
  align = mean_i ||qn_i - kn_i||^2 = 2 - 2*mean_i <qn_i, kn_i>
  lunif(x) = log( sum_{i<j} exp(4*<x_i,x_j> - 4) / npairs )   (unit-norm rows)
  out = align + (lunif(qn) + lunif(kn)) / 2

Sharding: the strict-upper pairwise sum is decomposed into 512x512 blocks of
the NxN gram matrix; each of the 8 cores covers 17 blocks (2 diagonal + 15
off-diagonal) via the rotation pairing, with inputs host-gathered so the
compiled program is SPMD-identical on every core.

v2 layout strategy: the host stages BOTH a transposed [D, rows] bf16 copy
(matmul operand layout — no on-device transposes at all) and a natural
[rows, D] bf16 copy (row-sumsq layout, tiled so each partition holds a
contiguous row range).  Device pipeline per chunk of rows:
  sumsq (DVE/GpSimd squares + fold-tree) -> rsqrt (DVE magic-Newton) ->
  flatten rn to a [1, n] row (tiny DMA) -> broadcast to [128, n] (GpSimd) ->
  normalize the transposed copy (DVE bf16 2x) -> gram matmuls (PE bf16) ->
  exp + reduce.
The exp of the 34 [128,2048] PSUM unit tiles is split across TWO engines:
~20 units on ACT (table exp, fused accumulate) and ~14 units on DVE via a
Schraudolph-style bit-trick exp (one tensor_scalar: bf16 bit pattern =
int16(s*738.66 + B)); those bf16 tiles are DMA'd to DRAM and summed on the
host (part of the unshard/all-reduce step).  The align term is one fused
multiply-reduce over the normalized slot-0/1 columns (each global row block
is covered exactly once across the 8 cores).
"""

import functools

import numpy as np

import concourse.bacc as bacc
import concourse.mybir as mybir
import concourse.tile as tile

# ----------------------------------------------------------------------------
# Problem constants (hardcoded per harness contract).
N = 8192
D = 128
NCORES = 8
NB = 16           # row blocks of the full N
BLK = 512
NSLOT = 11        # gathered blocks per core
GROWS = NSLOT * BLK   # 5632 gathered rows per core per tensor

# unit list: (row_slot, col_slot, is_diag) -- identical on every core.
UNITS = (
    [(0, 0, True), (1, 1, True)]
    + [(0, r, False) for r in range(1, 8)]
    + [(1, 1 + r, False) for r in range(1, 8)]
    + [(10, 9, False)]
)
NU = len(UNITS)  # 17

# chunk pipeline: (row0, row1, nat tiles per partition)
CHUNKS = [(0, 1024, 8), (1024, 3072, 16), (3072, 5632, 20)]
# ssq/rn16 compact col layout [128, 88]: per chunk, q seg then k seg
SSQ_SEG = {
    (0, 0): (0, 8), (1, 0): (8, 16),
    (0, 1): (16, 32), (1, 1): (32, 48),
    (0, 2): (48, 68), (1, 2): (68, 88),
}

# wave g = units whose largest slot falls inside chunk g's slots
WAVES = [[0, 1, 2], [3, 4, 5, 6, 9, 10, 11, 12], [7, 8, 13, 14, 15, 16]]
# 9 units take the DVE bit-exp path (offdiag only); rest go to ACT.
# Wave A stays all-ACT (the DVE is busy with the chunk-B/C chains then);
# the DVE share concentrates in waves B/C where the chains are done.
DVE_SET = {
    (0, 4), (1, 4), (0, 10), (1, 10),
    (0, 14), (1, 14), (0, 16), (1, 16), (0, 8),
}
# rn-broadcast pieces per chunk (PSUM outer-product tiles are <= 2048 wide)
BCAST_PIECES = [[(0, 1024)], [(1024, 3072)], [(3072, 5120), (5120, 5632)]]

# global schedule: (ti, u, kind); kind: 0 = ACT exp, 1 = DVE schraudolph
UNIT_SCHED = []
for _w in WAVES:
    for _u in _w:
        for _ti in range(2):
            UNIT_SCHED.append((_ti, _u, 1 if (_ti, _u) in DVE_SET else 0))
ACT_COL = {}
DVE_IDX = {}
for _ti, _u, _k in UNIT_SCHED:
    if _k == 0:
        ACT_COL[(_ti, _u)] = len(ACT_COL)
    else:
        DVE_IDX[(_ti, _u)] = len(DVE_IDX)
N_ACT = len(ACT_COL)   # 20
N_DVE = len(DVE_IDX)   # 14
ALIGN_COL = N_ACT      # accs col for the align accumulate
ACC_COLS = N_ACT + 1

# Schraudolph constants: bf16 bits of exp(4s-4) ~= int16(s*A + B).
# B assumes round-to-nearest fp32->int16 conversion and includes the
# arithmetic-mean-preserving correction sigma=log2(E[(1+f)2^-f])=0.05756.
SCH_A = 738.65988
SCH_B = 16256.0 - 738.65988 - 128.0 * 0.057567


DEBUG_DISABLE: set = set()  # bisect switches: gpsq, pbcast, ttr, schdma, schop


def _core_blocks(c: int) -> list[int]:
    """Row-block indices gathered for core c, slot order 0..10."""
    return [(2 * c + s) % NB for s in range(9)] + [(c + 8) % NB, c]


# ----------------------------------------------------------------------------
# Workaround: this walrus build rejects >1 semaphore wait per instruction, but
# TileContext's stock exit drain carries one wait per active proc.  Split it
# into one single-wait drain per proc.
def _apply_tile_exit_patch():
    import re

    import bass_rust
    from concourse.vector_clock import ScopedClock

    if getattr(tile.TileContext, "_drain_split_patch", False):
        return

    def _drain_and_barrier(self, tick_clock, wait_clock):
        nc = self.nc
        ticks = [int(s) for s in re.findall(r"\d+", repr(tick_clock.global_clock))]
        for p, t in ((p, t) for p, t in enumerate(ticks) if t > 0):
            vc = bass_rust.VectorClock()
            vc.require_at_least(p, t)
            d = nc.sync.drain()
            wait_clock.add_sem_waits(d.ins, ScopedClock({None: vc}))
        nc.all_engine_barrier()
        assert self.sems is not None
        popped = nc._tile_sem_poison_stack.pop()
        assert popped is self._sem_poison
        nc.clear_and_free_semaphores(list(self.sems.allocated().values()))
        nc.all_engine_barrier()

    tile.TileContext._drain_and_barrier = _drain_and_barrier
    tile.TileContext._drain_split_patch = True


# ----------------------------------------------------------------------------
def _emit(nc, tc, ctx, qt_d, kt_d, qn_d, kn_d, out_d, sch_d):
    f32 = mybir.dt.float32
    bf16 = mybir.dt.bfloat16
    i16 = mybir.dt.int16
    u32 = mybir.dt.uint32
    AF = mybir.ActivationFunctionType
    ALU = mybir.AluOpType

    big = ctx.enter_context(tc.tile_pool(name="big", bufs=1))
    scratch = ctx.enter_context(tc.tile_pool(name="scratch", bufs=2))
    psp = ctx.enter_context(tc.tile_pool(name="ps", bufs=2, space="PSUM"))

    t_d = (qt_d, kt_d)
    n_d = (qn_d, kn_d)

    xt = [big.tile([128, GROWS], bf16, tag=f"xt{ti}", name=f"xt{ti}") for ti in range(2)]
    xtn = [big.tile([128, GROWS], bf16, tag=f"xtn{ti}", name=f"xtn{ti}") for ti in range(2)]
    rnrow = [big.tile([1, GROWS], bf16, tag=f"rnrow{ti}", name=f"rnrow{ti}") for ti in range(2)]
    ones1 = big.tile([1, 512], bf16, tag="ones1")
    nc.vector.memset(ones1, 1.0)
    nat = [
        [big.tile([128, t, D], bf16, tag=f"nat{ti}_{g}", name=f"nat{ti}_{g}") for g, (_, _, t) in enumerate(CHUNKS)]
        for ti in range(2)
    ]
    ssq = big.tile([128, 88], f32, tag="ssq")
    rn = big.tile([128, 88], f32, tag="rn")
    rn16 = big.tile([128, 88], bf16, tag="rn16")
    accs = big.tile([128, ACC_COLS], f32, tag="accs")
    biasm4 = big.tile([128, 1], f32, tag="biasm4")
    nc.vector.memset(biasm4, -4.0)
    magic = big.tile([128, 1], u32, tag="magic")
    nc.vector.memset(magic, 0x5F3759DF)

    # ---- input DMAs, chunk A first so its chain starts early; halve each
    # chunk-A transfer so it spreads over more queues.
    for g, (r0, r1, t) in enumerate(CHUNKS):
        for ti in range(2):
            if g == 0:
                rm = (r0 + r1) // 2
                nc.sync.dma_start(
                    nat[ti][g][0:64, :, :],
                    n_d[ti][r0:rm].rearrange("(p t) d -> p t d", p=64),
                )
                nc.sync.dma_start(
                    nat[ti][g][64:128, :, :],
                    n_d[ti][rm:r1].rearrange("(p t) d -> p t d", p=64),
                )
                nc.sync.dma_start(xt[ti][:, r0:rm], t_d[ti][:, r0:rm])
                nc.sync.dma_start(xt[ti][:, rm:r1], t_d[ti][:, rm:r1])
            else:
                # nat first (it gates the chunk's sumsq chain), split in half
                tm = t // 2
                src = n_d[ti][r0:r1].rearrange("(p t) d -> p t d", p=128)
                nc.sync.dma_start(nat[ti][g][:, 0:tm, :], src[:, 0:tm, :])
                nc.sync.dma_start(nat[ti][g][:, tm:t, :], src[:, tm:t, :])
                nc.sync.dma_start(xt[ti][:, r0:r1], t_d[ti][:, r0:r1])

    def sumsq_chunk(ti, g, square_engine):
        """squares + fold tree + reduce -> ssq segment (compact f32)."""
        _, _, t = CHUNKS[g]
        s0, s1 = SSQ_SEG[(ti, g)]
        sq = scratch.tile([128, t, D], bf16, tag=f"sq{g}", name=f"sq{ti}_{g}")
        square_engine.tensor_tensor(sq[:], nat[ti][g][:], nat[ti][g][:], ALU.mult)
        f1 = scratch.tile([128, t, 64], bf16, tag=f"f1{g}", name=f"f1{ti}_{g}")
        nc.vector.tensor_tensor(f1[:], sq[:, :, 0:64], sq[:, :, 64:128], ALU.add)
        f2 = scratch.tile([128, t, 32], bf16, tag=f"f2{g}", name=f"f2{ti}_{g}")
        nc.vector.tensor_tensor(f2[:], f1[:, :, 0:32], f1[:, :, 32:64], ALU.add)
        nc.vector.tensor_reduce(ssq[:, s0:s1], f2[:], mybir.AxisListType.X, ALU.add)

    def newton_seg(c0, c1):
        """rn = 1/sqrt(ssq) on ssq cols [c0, c1): magic + 1 Newton step."""
        w = c1 - c0
        x = ssq[:, c0:c1]
        y = rn[:, c0:c1]
        yu = y.bitcast(u32)
        tmp = scratch.tile([128, w], f32, tag="nr_tmp")
        nc.vector.tensor_scalar(yu, x.bitcast(u32), 1, None, op0=ALU.logical_shift_right)
        nc.vector.tensor_tensor(yu, magic[:, 0:1].to_broadcast((128, w)), yu, ALU.subtract)
        nc.vector.tensor_tensor(tmp[:], y, y, ALU.mult)
        nc.vector.scalar_tensor_tensor(tmp[:], x, 0.5, tmp[:], ALU.mult, ALU.mult)
        nc.vector.tensor_scalar(tmp[:], tmp[:], -1.0, 1.5, op0=ALU.mult, op1=ALU.add)
        nc.vector.tensor_tensor(rn16[:, c0:c1], y, tmp[:], ALU.mult)

    def spread_chunk(ti, g):
        """compact rn16 -> [1,n] row -> PE outer-product broadcast into PSUM
        -> normalize xt straight from PSUM.  (GpSimd partition_broadcast is
        avoided -- GpSimd tensor ops starve the DVE on the shared SBUF port;
        stride-0-source DMAs degenerate to per-element descriptors; DMA
        doubling chains cost ~3us serial latency per hop.)"""
        r0, r1, t = CHUNKS[g]
        s0, s1 = SSQ_SEG[(ti, g)]
        # Chunk A flattens ride the Activation DGE: its rings are empty while
        # the SP rings still hold megabytes of queued input loads.  Later
        # chunks flip: the ACT queue is packed with exps by then (a trigger
        # there waits ~15us), while the SP rings have drained.
        dge = nc.scalar if g == 0 else nc.sync
        dge.dma_start(
            rnrow[ti][0:1, r0:r1].rearrange("o (p t) -> o p t", p=128),
            rn16[:, s0:s1],
        )
        for c0, c1 in BCAST_PIECES[g]:
            w = c1 - c0
            rnp = psp.tile([128, 2048], f32, tag="ps", name=f"rnp{ti}_{g}_{c0}")
            for m0 in range(0, w, 512):
                m1 = min(m0 + 512, w)
                nc.tensor.matmul(
                    rnp[:, m0:m1],
                    lhsT=ones1[:, 0:128],
                    rhs=rnrow[ti][0:1, c0 + m0 : c0 + m1],
                    start=True,
                    stop=True,
                )
            nc.vector.tensor_tensor(
                xtn[ti][:, c0:c1], xt[ti][:, c0:c1], rnp[:, 0:w], ALU.mult
            )

    def emit_unit(ti, u):
        rs, cs, _ = UNITS[u]
        ps = psp.tile([128, 2048], f32, tag="ps", name=f"ps{ti}_{u}")
        for m in range(4):
            nc.tensor.matmul(
                ps[:, 512 * m : 512 * (m + 1)],
                lhsT=xtn[ti][:, BLK * rs + 128 * m : BLK * rs + 128 * (m + 1)],
                rhs=xtn[ti][:, BLK * cs : BLK * (cs + 1)],
                start=True,
                stop=True,
            )
        if (ti, u) in ACT_COL:
            col = ACT_COL[(ti, u)]
            ad = scratch.tile([128, 2048], bf16, tag="actdump")
            nc.scalar.activation(
                ad[:], ps[:], AF.Exp, bias=biasm4[:], scale=4.0,
                accum_out=accs[:, col : col + 1],
            )
        else:
            idx = DVE_IDX[(ti, u)]
            sch = scratch.tile([128, 2048], i16, tag="sch")
            if "schop" in DEBUG_DISABLE:
                nc.vector.tensor_scalar(
                    sch[:].bitcast(bf16), ps[:], 1.0, None, op0=ALU.mult
                )
            else:
                nc.vector.tensor_scalar(
                    sch[:], ps[:], SCH_A, SCH_B, op0=ALU.mult, op1=ALU.add
                )
            if "schdma" not in DEBUG_DISABLE:
                nc.sync.dma_start(sch_d[idx], sch[:].bitcast(bf16))

    # ---- PE warm-up: dummy K=1 matmuls reading the freshly-landed xt tile
    # keep HAM busy from the moment inputs arrive until the first real grams,
    # so those run at the unthrottled clock.
    dps = psp.tile([128, 2048], f32, tag="ps", name="dummyps")
    for m in range(8):
        nc.tensor.matmul(
            dps[:, 512 * (m % 4) : 512 * (m % 4 + 1)],
            lhsT=ones1[:, 0:128],
            rhs=xt[0][0:1, 0:512],
            start=True,
            stop=True,
        )

    # ---- chunk A, per tensor: fastest possible path to the first exps
    for ti in range(2):
        sumsq_chunk(ti, 0, nc.vector)
        newton_seg(*SSQ_SEG[(ti, 0)])
        spread_chunk(ti, 0)
        for u in WAVES[0]:
            emit_unit(ti, u)

    # ---- chunk B chain, then wave B first half (chunk C chain mid-wave)
    for ti in range(2):
        sumsq_chunk(ti, 1, nc.vector)
    newton_seg(16, 48)
    for ti in range(2):
        spread_chunk(ti, 1)

    WB = [(0, 3), (1, 3), (0, 4), (0, 5), (1, 5), (1, 4), (0, 6), (1, 6),
          (0, 10), (0, 9), (1, 9), (1, 10), (0, 11), (1, 11), (0, 12), (1, 12)]
    for ti, u in WB[:8]:
        emit_unit(ti, u)

    for ti in range(2):
        sumsq_chunk(ti, 2, nc.vector)
    newton_seg(48, 88)

    for ti, u in WB[8:12]:
        emit_unit(ti, u)

    for ti in range(2):
        spread_chunk(ti, 2)

    for ti, u in WB[12:]:
        emit_unit(ti, u)

    # align term: sum <qn_i, kn_i> over slots 0-1 rows (once per row globally)
    aldump = scratch.tile([128, 1024], bf16, tag="aldump")
    nc.vector.scalar_tensor_tensor(
        aldump[:], xtn[0][:, 0:1024], 1.0, xtn[1][:, 0:1024], ALU.mult, ALU.mult,
        accum_out=accs[:, ALIGN_COL : ALIGN_COL + 1],
    )

    WC = [(0, 7), (0, 8), (1, 7), (0, 14), (0, 13), (1, 14), (1, 13),
          (0, 16), (0, 15), (1, 16), (1, 15), (1, 8)]
    for ti, u in WC:
        emit_unit(ti, u)

    nc.sync.dma_start(out_d[:], accs[:])


@functools.lru_cache(maxsize=1)
def _build():
    from contextlib import ExitStack

    _apply_tile_exit_patch()
    nc = bacc.Bacc("TRN2", target_bir_lowering=False, debug=False, num_devices=NCORES)
    f32 = mybir.dt.float32
    bf16 = mybir.dt.bfloat16
    i16 = mybir.dt.int16
    qt = nc.dram_tensor("qt", [D, GROWS], bf16, kind="ExternalInput")
    kt = nc.dram_tensor("kt", [D, GROWS], bf16, kind="ExternalInput")
    qn = nc.dram_tensor("qn", [GROWS, D], bf16, kind="ExternalInput")
    kn = nc.dram_tensor("kn", [GROWS, D], bf16, kind="ExternalInput")
    out = nc.dram_tensor("out", [128, ACC_COLS], f32, kind="ExternalOutput")
    sch = nc.dram_tensor("sch", [N_DVE, 128, 2048], bf16, kind="ExternalOutput")
    with tile.TileContext(nc) as tc, ExitStack() as ctx:
        _emit(nc, tc, ctx, qt.ap(), kt.ap(), qn.ap(), kn.ap(), out.ap(), sch.ap())
    nc.compile()
    return nc


def _bf16(x: np.ndarray):
    import ml_dtypes

    return np.ascontiguousarray(x).astype(ml_dtypes.bfloat16)


def _stage(x: np.ndarray, c: int):
    """Gather core c's row blocks; return (transposed bf16, natural bf16)."""
    g = np.concatenate([x[BLK * b : BLK * (b + 1)] for b in _core_blocks(c)])
    return _bf16(g.T), _bf16(g)


def run_device(q: np.ndarray, k: np.ndarray, **run_kwargs):
    """Compile + run on the 8 cores; returns BassKernelResults."""
    from concourse.bass_utils import run_bass_kernel_spmd

    nc = _build()
    in_maps = []
    for c in range(NCORES):
        qt, qn = _stage(q, c)
        kt, kn = _stage(k, c)
        in_maps.append({"qt": qt, "kt": kt, "qn": qn, "kn": kn})
    return run_bass_kernel_spmd(nc, in_maps, core_ids=list(range(NCORES)), **run_kwargs)


def reduce_outputs(outs: list) -> np.float32:
    """Host-side gather/unshard: fold per-core accumulators into the scalar."""
    npairs = N * (N - 1) / 2.0
    diag = [0.0, 0.0]
    off = [0.0, 0.0]
    align_dot = 0.0
    for c in range(NCORES):
        acc = outs[c]["out"].astype(np.float64)
        for (ti, u), col in ACT_COL.items():
            s = acc[:, col].sum()
            if UNITS[u][2]:
                diag[ti] += s
            else:
                off[ti] += s
        align_dot += acc[:, ALIGN_COL].sum()
        schf = np.asarray(outs[c]["sch"]).astype(np.float64)
        for (ti, u), idx in DVE_IDX.items():
            off[ti] += schf[idx].sum()
    terms = [np.log((off[ti] + (diag[ti] - N) / 2.0) / npairs) for ti in range(2)]
    align = 2.0 - 2.0 * align_dot / N
    return np.float32(align + (terms[0] + terms[1]) / 2.0)


def kernel(q: np.ndarray, k: np.ndarray) -> np.ndarray:
    res = run_device(q, k)
    return np.asarray(reduce_outputs(res.results), dtype=np.float32)
